# revision 37
# baseline (speedup 1.0000x reference)
"""Trainium2 Bass kernel for CustomMultiHeadAttention (RoPE + causal MHA).

Sharding: 8 cores = 2 batches x 4 head-groups (4 heads each).
Each core computes, for its (batch, head-group):
  QT/KT = (Wq|Wk col-slice, half-permuted).T @ xT   -> [256, S] feature-major
  RoPE on QT/KT (full-tile DVE ops thanks to half-grouped layout)
  V = xT.T @ Wv col-slice                            -> [S, 256] natural
  per head: scoresT[k,q] = KT_h.T @ QT_h (row-packed over heads, K=64 pairs)
            expT = exp(scoresT/8) (no max subtraction; scores are bounded)
            causal: skip blocks above diagonal, 0/1-mask diagonal blocks
            ctxT[d,q] (+denominator row via ones-column in V_aug) = V_aug.T @ expT
  normalize ctxT by the denominator row (reciprocal_approx_fast + DVE mult)
  partial_out = ctxT.T @ Wo row-slice               -> [S, 1024] (bf16)
Host: sums the 4 head-group partials per batch, adds bo.

v2 structure (vs v1): bf16 staging for x/W/out, pair-interleaved flash
attention (PSUM: 4 ctx banks + 2 score banks + 2 projection banks), one-chunk
software pipeline (projections for chunk qc+1 fill PE gaps while attention qc
is ACT(exp)-bound), reciprocal_approx_fast for the softmax denominators,
batched input/output DMAs.
"""

import os
import sys

for _p in ("/opt/trn_rl_repo", "/root/.axon_site/_ro/trn_rl_repo"):
    if os.path.isdir(_p) and _p not in sys.path:
        sys.path.insert(0, _p)

import numpy as np
import ml_dtypes

import concourse.bass as bass
import concourse.bacc as bacc
import concourse.mybir as mybir
import concourse.tile as tile
from concourse.bass_utils import run_bass_kernel_spmd

F32 = mybir.dt.float32
BF16 = mybir.dt.bfloat16
AF = mybir.ActivationFunctionType
ALU = mybir.AluOpType

NUM_HEADS = 16
HD = 64
D = NUM_HEADS * HD  # 1024
B = 2
S = 2048
NCORES = 8
HPC = 4            # heads per core
JC = HPC * HD      # 256 per-core projection width
P = 128


def build_core(tc, io, s_len=S):
    """Emit the per-core program. io: dict of DRAM APs. s_len: sequence length
    (parameterized so the simulator self-test can run a smaller size)."""
    nc = tc.nc
    SL = s_len
    NST = SL // P          # 128-row seq tiles
    NQC = SL // 512        # 512-wide q chunks
    NDT = D // P           # 8 k-tiles over d_model
    scale = 1.0 / np.sqrt(HD)

    xT_d, wq_d, wk_d, wv_d, wo_d = io["xT"], io["wq"], io["wk"], io["wv"], io["wo"]
    sin_d, cos_d, mask_d, out_d = io["sin"], io["cos"], io["mask"], io["out"]

    import contextlib
    with contextlib.ExitStack() as ctx:
        cpool = ctx.enter_context(tc.tile_pool(name="const", bufs=1))
        epool = ctx.enter_context(tc.tile_pool(name="expt", bufs=8))
        tpool = ctx.enter_context(tc.tile_pool(name="tmps", bufs=6))
        ps_ctx = ctx.enter_context(tc.tile_pool(name="ps_ctx", bufs=2, space="PSUM"))
        ps_sc = ctx.enter_context(tc.tile_pool(name="ps_sc", bufs=2, space="PSUM"))
        ps_proj = ctx.enter_context(tc.tile_pool(name="ps_pr", bufs=2, space="PSUM"))

        # ---- persistent SBUF tensors ----
        xT = cpool.tile([P, NDT, SL], BF16, tag="xT")
        wq = cpool.tile([P, NDT, JC], BF16, tag="wq")
        wk = cpool.tile([P, NDT, JC], BF16, tag="wk")
        wv = cpool.tile([P, NDT, JC], BF16, tag="wv")
        wo = cpool.tile([P, 2, D], BF16, tag="wo")
        sinf = cpool.tile([P, SL], F32, tag="sinf")
        cosf = cpool.tile([P, SL], F32, tag="cosf")
        mask = cpool.tile([P, P], BF16, tag="mask")
        QT = cpool.tile([P, 2, SL], BF16, tag="QT")     # plane 0: first halves
        KT = cpool.tile([P, 2, SL], BF16, tag="KT")
        # head-contiguous copies: plane p holds heads 2p (parts 0-63) and
        # 2p+1 (parts 64-127), dims 0-31 = rotated first half, 32-63 second
        QTc = cpool.tile([P, 2, SL], BF16, tag="QTc")
        KTc = cpool.tile([P, 2, SL], BF16, tag="KTc")
        V = cpool.tile([P, NST, HPC, 2 * HD], BF16, tag="V")
        ctxA = cpool.tile([P, SL], BF16, tag="ctxA")     # heads 0,1 (j on partitions)
        ctxB = cpool.tile([P, SL], BF16, tag="ctxB")     # heads 2,3

        def xt_load(qc_i):
            nc.sync.dma_start(
                xT[:, :, qc_i * 512:(qc_i + 1) * 512],
                xT_d.rearrange("(t p) s -> p t s", p=P)[:, :, qc_i * 512:(qc_i + 1) * 512])

        # input DMAs. First k-tile slivers of wq/xT go on their own small
        # DMAs so the very first matmul only waits ~192KB, not 3MB.
        wq_r = wq_d.rearrange("(t p) j -> p t j", p=P)
        xt_r = xT_d.rearrange("(t p) s -> p t s", p=P)
        nc.sync.dma_start(xT[:, 0:1, 0:512], xt_r[:, 0:1, 0:512])
        nc.sync.dma_start(wq[:, 0:1], wq_r[:, 0:1])
        nc.sync.dma_start(wq[:, 1:8], wq_r[:, 1:8])
        nc.sync.dma_start(wk[:], wk_d.rearrange("(t p) j -> p t j", p=P))
        # xt_rest split in two: dt1-3 matmuls start mid-transfer, and the
        # PE wait stays under the ~3.4us HAM re-throttle window
        nc.sync.dma_start(xT[:, 1:4, 0:512], xt_r[:, 1:4, 0:512])
        nc.sync.dma_start(xT[:, 4:8, 0:512], xt_r[:, 4:8, 0:512])
        nc.sync.dma_start(sinf[:], sin_d[:])
        nc.sync.dma_start(cosf[:], cos_d[:])
        nc.sync.dma_start(wv[:], wv_d.rearrange("(t p) j -> p t j", p=P))
        nc.sync.dma_start(mask[:], mask_d[:])
        # denominator ones columns first (den lands at psum partitions 0-63:
        # reciprocal_approx_fast drops nonzero base partitions, and PSUM
        # reads must be 32-partition-aligned)
        nc.gpsimd.memset(V[:, :, :, 0:HD], 1.0)

        def qk_proj(qc, w_sb, out_sb):
            """Project + RoPE one 512-col s-chunk of QT or KT."""
            sl = slice(qc * 512, qc * 512 + 512)
            pps = []
            for jt in range(2):
                pp = ps_proj.tile([P, 512], F32, tag="proj")
                for dt_i in range(NDT):
                    nc.tensor.matmul(
                        pp[:],
                        lhsT=w_sb[:, dt_i, jt * P:(jt + 1) * P],
                        rhs=xT[:, dt_i, sl],
                        start=(dt_i == 0), stop=(dt_i == NDT - 1),
                    )
                pps.append(pp)
            pA, pB = pps
            csl, ssl = cosf[:, sl], sinf[:, sl]
            # bf16 temporaries: the combine ops become all-bf16-SBUF (DVE 2x)
            t1 = tpool.tile([P, 512], BF16, tag="t1")
            t2 = tpool.tile([P, 512], BF16, tag="t2")
            nc.vector.tensor_tensor(t1[:], pA[:], csl, ALU.mult)
            nc.vector.tensor_tensor(t2[:], pB[:], ssl, ALU.mult)
            nc.vector.tensor_tensor(out_sb[:, 0, sl], t1[:], t2[:], ALU.subtract)
            t3 = tpool.tile([P, 512], BF16, tag="t1")
            t4 = tpool.tile([P, 512], BF16, tag="t2")
            nc.vector.tensor_tensor(t3[:], pA[:], ssl, ALU.mult)
            nc.vector.tensor_tensor(t4[:], pB[:], csl, ALU.mult)
            nc.vector.tensor_tensor(out_sb[:, 1, sl], t3[:], t4[:], ALU.add)
            dst = QTc if out_sb is QT else KTc
            for h in range(HPC):
                for half in range(2):
                    nc.sync.dma_start(
                        dst[64 * (h % 2) + 32 * half:64 * (h % 2) + 32 * half + 32,
                            h // 2, sl],
                        out_sb[32 * h:32 * h + 32, half, sl])

        def v_proj(st):
            """Project one 128-row seq tile of V into V_sb (strided, +1 col)."""
            ppf = ps_proj.tile([P, 512], F32, tag="proj", name="vproj")
            pp = ppf[:, :JC]
            for dt_i in range(NDT):
                nc.tensor.matmul(
                    pp[:],
                    lhsT=xT[:, dt_i, st * P:(st + 1) * P],
                    rhs=wv[:, dt_i, :],
                    start=(dt_i == 0), stop=(dt_i == NDT - 1),
                )
            nc.vector.tensor_copy(
                out=V[:, st, :, HD:2 * HD],
                in_=pp[:].rearrange("p (h d) -> p h d", h=HPC),
            )

        def attention(qc):
            """Causal flash attention for one q-chunk, two passes (head
            pairs). Both heads of a pair share one 2-bank score tile so a
            single wide exp serves both; the two ctx matmuls then have one
            shared dependency and issue back-to-back."""
            n_ki = 4 * qc + 4
            for pair in range(2):
                heads = (2 * pair, 2 * pair + 1)
                ctx_ps = [ps_ctx.tile([P, 512], F32, tag="ctx",
                                      name=f"ctx{h}")
                          for h in heads]
                for ki in range(n_ki):
                    diag_r = ki - 4 * qc
                    c0 = 128 * diag_r if diag_r >= 0 else 0
                    nv = 512 - c0
                    qsl = slice(qc * 512 + c0, qc * 512 + 512)
                    st_ps = ps_sc.tile([P, 1024], F32, tag="sc")
                    for hh in range(2):
                        nc.tensor.matmul(
                            st_ps[:, 512 * hh:512 * hh + nv],
                            lhsT=KTc[64 * hh:64 * hh + 64, pair,
                                     ki * P:(ki + 1) * P],
                            rhs=QTc[64 * hh:64 * hh + 64, pair, qsl],
                            start=True, stop=True,
                            tile_position=(64 * hh, 0),
                        )
                    et = epool.tile([P, 1024], BF16, tag="expT")
                    if diag_r <= 0:
                        # nv == 512: one exp covers both heads contiguously
                        nc.scalar.activation(et[:], st_ps[:], AF.Exp,
                                             scale=float(scale))
                    else:
                        for hh in range(2):
                            nc.scalar.activation(
                                et[:, 512 * hh:512 * hh + nv],
                                st_ps[:, 512 * hh:512 * hh + nv],
                                AF.Exp, scale=float(scale))
                    if diag_r >= 0:
                        nc.gpsimd.tensor_tensor(et[:, 0:P], et[:, 0:P], mask[:],
                                                ALU.mult)
                        nc.gpsimd.tensor_tensor(et[:, 512:512 + P],
                                                et[:, 512:512 + P], mask[:],
                                                ALU.mult)
                    for hh, h in enumerate(heads):
                        nc.tensor.matmul(
                            ctx_ps[hh][:, c0:512],
                            lhsT=V[:, ki, h, :],
                            rhs=et[:, 512 * hh:512 * hh + nv],
                            start=(ki == 0), stop=(ki == n_ki - 1),
                        )
                # normalize: psum rows 0-63 hold the denominator (ctx 64-127)
                for hh, h in enumerate(heads):
                    denb = tpool.tile([HD, 512], F32, tag="denb")
                    nc.vector.reciprocal_approx_fast(denb[:], ctx_ps[hh][0:HD, :])
                    dst = ctxA if h < 2 else ctxB
                    nc.vector.tensor_tensor(
                        dst[HD * (h % 2):HD * (h % 2) + HD,
                            qc * 512:qc * 512 + 512],
                        ctx_ps[hh][HD:2 * HD, :], denb[:], ALU.mult)

        def out_proj(st):
            ot = tpool.tile([P, D], BF16, tag="ostage")
            for nh in range(2):
                pp = ps_proj.tile([P, 512], F32, tag="proj", name="oproj")
                for jt, csb in enumerate((ctxA, ctxB)):
                    nc.tensor.matmul(
                        pp[:],
                        lhsT=csb[:, st * P:(st + 1) * P],
                        rhs=wo[:, jt, nh * 512:nh * 512 + 512],
                        start=(jt == 0), stop=(jt == 1),
                    )
                nc.vector.tensor_copy(out=ot[:, nh * 512:nh * 512 + 512], in_=pp[:])
            nc.sync.dma_start(out_d[st * P:(st + 1) * P, :], ot[:])

        # ---- emission (priority) order: 1-chunk software pipeline ----
        # prologue: chunk 0 projections
        qk_proj(0, wq, QT)
        qk_proj(0, wk, KT)
        for st in range(4):
            v_proj(st)
        if NQC > 1:
            xt_load(1)
        nc.sync.dma_start(wo[:], wo_d.rearrange("(t p) n -> p t n", p=P))
        for qc in range(NQC):
            attention(qc)
            if qc + 1 < NQC:
                qk_proj(qc + 1, wq, QT)
                qk_proj(qc + 1, wk, KT)
                for st in range(4 * qc + 4, 4 * qc + 8):
                    v_proj(st)
            if qc + 2 < NQC:
                xt_load(qc + 2)
            for st in range(4 * qc, 4 * qc + 4):
                out_proj(st)


# ----------------------------------------------------------------------------
# host side
# ----------------------------------------------------------------------------

def _rope_tables(s_len):
    pos = np.arange(s_len, dtype=np.float32)
    inv_freq = np.exp(np.arange(0, HD, 2, dtype=np.float32)
                      * (-np.log(10000.0) / HD)).astype(np.float32)
    ang = pos[:, None] * inv_freq[None, :]          # [S, 32]
    sin = np.sin(ang).astype(np.float32)
    cos = np.cos(ang).astype(np.float32)
    # [128, S]: row 32h + i = table for freq i, replicated over the 4 heads
    sinf = np.ascontiguousarray(np.tile(sin.T, (HPC, 1)))
    cosf = np.ascontiguousarray(np.tile(cos.T, (HPC, 1)))
    return sinf, cosf


def _half_perm():
    """Column permutation grouping first/second halves of the 4 heads."""
    first = [64 * h + d for h in range(HPC) for d in range(32)]
    second = [64 * h + d for h in range(HPC) for d in range(32, 64)]
    return np.array(first + second, dtype=np.int64)


def build_program(s_len=S):
    nc = bacc.Bacc("TRN2", target_bir_lowering=False, debug=False,
                   num_devices=NCORES)
    io = {
        "xT": nc.dram_tensor("xT", [D, s_len], BF16, kind="ExternalInput").ap(),
        "wq": nc.dram_tensor("wq", [D, JC], BF16, kind="ExternalInput").ap(),
        "wk": nc.dram_tensor("wk", [D, JC], BF16, kind="ExternalInput").ap(),
        "wv": nc.dram_tensor("wv", [D, JC], BF16, kind="ExternalInput").ap(),
        "wo": nc.dram_tensor("wo", [JC, D], BF16, kind="ExternalInput").ap(),
        "sin": nc.dram_tensor("sin", [P, s_len], F32, kind="ExternalInput").ap(),
        "cos": nc.dram_tensor("cos", [P, s_len], F32, kind="ExternalInput").ap(),
        "mask": nc.dram_tensor("mask", [P, P], BF16, kind="ExternalInput").ap(),
        "out": nc.dram_tensor("out", [s_len, D], BF16, kind="ExternalOutput").ap(),
    }
    with tile.TileContext(nc) as tc:
        build_core(tc, io, s_len)
    nc.compile()
    return nc


def make_in_maps(x, Wq, Wk, Wv, Wo, s_len=S):
    """Shard the full inputs into one input map per core."""
    perm = _half_perm()
    sinf, cosf = _rope_tables(s_len)
    mask = np.triu(np.ones((P, P), dtype=np.float32)).astype(ml_dtypes.bfloat16)
    bf = ml_dtypes.bfloat16
    in_maps = []
    for c in range(NCORES):
        b, g = divmod(c, NCORES // B)
        cols = slice(JC * g, JC * (g + 1))
        in_maps.append({
            "xT": np.ascontiguousarray(x[b].T).astype(bf),
            "wq": np.ascontiguousarray(Wq[:, cols][:, perm]).astype(bf),
            "wk": np.ascontiguousarray(Wk[:, cols][:, perm]).astype(bf),
            "wv": np.ascontiguousarray(Wv[:, cols]).astype(bf),
            "wo": np.ascontiguousarray(Wo[cols, :]).astype(bf),
            "sin": sinf, "cos": cosf, "mask": mask,
        })
    return in_maps


_CACHED_NC = None


def kernel(x, Wq, bq, Wk, bk, Wv, bv, Wo, bo, **run_kwargs):
    global _CACHED_NC
    x, Wq, bq, Wk, bk, Wv, bv, Wo, bo = (
        np.asarray(a, dtype=np.float32)
        for a in (x, Wq, bq, Wk, bk, Wv, bv, Wo, bo))
    assert not (np.any(bq) or np.any(bk) or np.any(bv)), \
        "nonzero qkv biases not supported by this build"
    if _CACHED_NC is None:
        _CACHED_NC = build_program(S)
    in_maps = make_in_maps(x, Wq, Wk, Wv, Wo, S)
    res = run_bass_kernel_spmd(_CACHED_NC, in_maps, list(range(NCORES)),
                               **run_kwargs)
    out = np.zeros((B, S, D), dtype=np.float32)
    for c in range(NCORES):
        b = c // (NCORES // B)
        out[b] += np.asarray(res.results[c]["out"], dtype=np.float32)
    out += bo[None, None, :]
    if run_kwargs:
        kernel.last_result = res
    return out
